# revision 9
# baseline (speedup 1.0000x reference)
"""Trainium2 Bass kernel for nn_ExtractorMLP (GNN edge cosine-similarity).

Math:  out[e] = cos_sim(mlp(emb[col[e]]), mlp(emb[row[e]]))
where  mlp(x) = elu(x @ W1.T + b1) @ W2.T + b2   (b1 = b2 = 0 here)

v2 strategy (edge-data-parallel across 8 cores, one SPMD program):
  * Phase 1 (per node, replicated): UNnormalized MLP table t[v] (bf16) in
    node-major layout ([v%128 partition, v//128 stripe of 128 feats]),
    kept in SBUF (row-side dma_gather source) and mirrored to DRAM
    (col-side dynamic block loads).  Per-node sum-of-squares goes to the
    host, which applies the cosine normalization at the end.
  * Phase 2 (edges, sharded): per core, edges are bucketed by row-half
    (int16 gather-index limit), col-sorted within each bucket, and cut
    into 384-edge chunks.
      - col side (NO gather): each chunk touches <=4 distinct 128-node
        table blocks.  Block row-offsets ship as data and are loaded with
        dynamic-offset DMAs (reg_load + snap + bass.ds); one-hots
        [128, 384] are built on DVE with is_equal against host codes
        (128*slot + col%128) vs iota columns, and
        f1 = sum_s block_s.T @ onehot_s via accumulating matmuls.
      - row side: SWDGE dma_gather (transpose, SBUF source), 7680 edges
        per instruction; -1 index tails are trimmed by the ucode at zero
        descriptor-gen cost.
      - dot: DVE multiply f1(PSUM) * f2(SBUF), then contraction over the
        128 feature partitions with the sliding one-hot matmul trick,
        128 chunks per PSUM output group.
  * Host: out[e] = dots[e] / (n[col] * n[row]), n = max(sqrt(ssq), eps).
"""

import math

import numpy as np
import ml_dtypes

BF16 = ml_dtypes.bfloat16
F16 = np.float16

H = 128            # feature dim
P = 128            # partitions
CHUNK = 384        # edges per expansion chunk
NSLOT = 4          # table blocks per chunk
BATCH = 7680       # edges per dma_gather instruction / f2 tile (20 chunks)
CPB = BATCH // CHUNK
HALF = 32768       # int16 index limit: row-bucket split
NCORES = 8
ST_W = 512         # phase-1 supertile width (nodes)
PAD_CODE = 1000.0  # code value matching no slot

_PROG_CACHE: dict = {}
LAST_RESULTS = None


# --------------------------------------------------------------------------
# host-side packing
# --------------------------------------------------------------------------

def _wrap_idx(idx):
    """[S*16] int16 -> [128, S] wrapped layout (16 partitions, replicated 8x)."""
    w = idx.reshape(-1, 16).T.astype(np.int16)
    return np.tile(w, (8, 1))


def _pack_region(cols, rows, row_off):
    """Pack one row-half bucket (already col-sorted) into CHUNK streams.

    Returns (codes f32[S], ridx int64[S], meta u32[nck*NSLOT], pos int64[S]);
    pos[i] indexes into cols/rows for stream slot i (-1 = pad).  Chunk-
    internal pad gather idxs are 0 (safe valid index; f1=0 kills them),
    and converted to -1 only in the region tail by _pad_region.
    """
    n = len(cols)
    codes, ridx, meta, pos = [], [], [], []
    i = 0
    while i < n:
        blocks = {}
        c_codes = np.full(CHUNK, PAD_CODE, dtype=np.float32)
        c_ridx = np.zeros(CHUNK, dtype=np.int64)
        c_pos = np.full(CHUNK, -1, dtype=np.int64)
        j = 0
        while j < CHUNK and i < n:
            b = int(cols[i]) >> 7
            if b not in blocks:
                if len(blocks) == NSLOT:
                    break          # close chunk early (rare)
                blocks[b] = len(blocks)
            s = blocks[b]
            c_codes[j] = 128 * s + (int(cols[i]) & 127)
            c_ridx[j] = rows[i] - row_off
            c_pos[j] = i
            i += 1
            j += 1
        m = np.zeros(NSLOT, dtype=np.uint32)
        for b, s in blocks.items():
            m[s] = b * 128          # row offset into DRAM table
        codes.append(c_codes)
        ridx.append(c_ridx)
        meta.append(m)
        pos.append(c_pos)
    if not codes:
        return (np.zeros(0, np.float32), np.zeros(0, np.int64),
                np.zeros(0, np.uint32), np.zeros(0, np.int64))
    return (np.concatenate(codes), np.concatenate(ridx),
            np.concatenate(meta), np.concatenate(pos))


def _prep_core(col, row):
    """Bucket by row-half, col-sort, pack chunks."""
    lo_sel = row < HALF
    out = {}
    for name, sel, roff in (("lo", lo_sel, 0), ("hi", ~lo_sel, HALF)):
        c, r = col[sel], row[sel]
        orig = np.nonzero(sel)[0]
        order = np.argsort(c, kind="stable")
        c, r, orig = c[order], r[order], orig[order]
        codes, ridx, meta, pos = _pack_region(c, r, roff)
        opos = np.where(pos >= 0, orig[np.clip(pos, 0, None)], -1)
        out[name] = (codes, ridx, meta, opos)
    return out


def _pad_region(codes, ridx, meta, opos, cap, shared_real=None):
    """Pad a region's streams to cap edges (cap % CHUNK == 0).

    Gather idxs in [len, shared_real_of_batch) are 0 (safe pads shared
    across cores so every core trims at the same point); -1 beyond
    (trimmed by the gather ucode at zero descriptor cost)."""
    s = len(codes)
    assert s <= cap and cap % CHUNK == 0
    pc = np.full(cap, PAD_CODE, dtype=np.float32)
    pr = np.full(cap, -1, dtype=np.int64)
    po = np.full(cap, -1, dtype=np.int64)
    pm = np.zeros(cap // CHUNK * NSLOT, dtype=np.uint32)
    pc[:s] = codes
    pr[:s] = ridx
    po[:s] = opos
    pm[:len(meta)] = meta
    if shared_real is not None:
        for b0 in range(0, cap, BATCH):
            cnt = shared_real.get(b0, 0)
            lo = max(s - b0, 0)
            if cnt > lo:
                pr[b0 + lo:b0 + cnt] = 0
    return pc, pr, pm, po


# --------------------------------------------------------------------------
# device program
# --------------------------------------------------------------------------

def _build_program(n_pad, n_chunks, gathers, trace_label=""):
    """Build the shared SPMD program.

    gathers: list of (stream_pos, n_idx, half_id) gather instructions;
             each lies within one BATCH-aligned f2 tile.
    """
    import concourse.bacc as bacc
    import concourse.bass as bass
    import concourse.mybir as mybir
    import concourse.tile as tile
    from concourse import library_config
    from contextlib import ExitStack

    f32 = mybir.dt.float32
    bf16 = mybir.dt.bfloat16
    fp16 = mybir.dt.float16
    i16 = mybir.dt.int16
    u32 = mybir.dt.uint32
    Alu = mybir.AluOpType
    Act = mybir.ActivationFunctionType

    S = n_chunks * CHUNK
    n_groups = math.ceil(n_chunks / P)
    n_blocks = n_pad // H
    half = min(HALF, n_pad)
    n_batches = S // BATCH

    nc = bacc.Bacc("TRN2", target_bir_lowering=False, debug=False,
                   num_devices=NCORES, num_swdge_queues=1)

    embT = nc.dram_tensor("embT", [P, n_pad], bf16, kind="ExternalInput")
    w1t_d = nc.dram_tensor("w1t", [H, H], bf16, kind="ExternalInput")
    w2t_d = nc.dram_tensor("w2t", [H, H], bf16, kind="ExternalInput")
    iota_d = nc.dram_tensor("iota4", [P, NSLOT], f32, kind="ExternalInput")
    codes_d = nc.dram_tensor("codes", [P, S], fp16, kind="ExternalInput")
    ridx_d = nc.dram_tensor("ridx", [P, S // 16], i16, kind="ExternalInput")
    meta_d = nc.dram_tensor("meta", [1, n_chunks * NSLOT], u32,
                            kind="ExternalInput")
    tbl_d = nc.dram_tensor("tbl_dram", [n_pad, H], bf16, kind="ExternalOutput")
    ssq_d = nc.dram_tensor("ssq", [P, n_blocks], f32, kind="ExternalOutput")
    out_d = nc.dram_tensor("out", [n_groups, P, CHUNK], f32,
                           kind="ExternalOutput")

    # group gathers by f2 batch tile
    g_by_batch = [[] for _ in range(n_batches)]
    for (pos0, n_reg, hf) in gathers:
        bt = pos0 // BATCH
        assert pos0 % BATCH == 0 and 0 < n_reg <= BATCH
        g_by_batch[bt].append((pos0, n_reg, hf))

    with ExitStack() as ctx:
        tc = ctx.enter_context(tile.TileContext(nc))
        const = ctx.enter_context(tc.tile_pool(name="const", bufs=1))
        p1 = ctx.enter_context(tc.tile_pool(name="p1", bufs=3))
        pcode = ctx.enter_context(tc.tile_pool(name="pcode", bufs=3))
        pslot = ctx.enter_context(tc.tile_pool(name="pslot", bufs=2 * NSLOT))
        poh = ctx.enter_context(tc.tile_pool(name="poh", bufs=2 * NSLOT))
        pf2 = ctx.enter_context(tc.tile_pool(name="pf2", bufs=2))
        pprod = ctx.enter_context(tc.tile_pool(name="pprod", bufs=4))
        psA = ctx.enter_context(tc.tile_pool(name="psA", bufs=4, space="PSUM"))
        psB = ctx.enter_context(tc.tile_pool(name="psB", bufs=4, space="PSUM"))

        nc.gpsimd.load_library(library_config.mlp)

        # --- constants / persistent tiles ---
        table = const.tile([P, n_pad], bf16, tag="table")
        w1t = const.tile([H, H], bf16, tag="w1t")
        w2t = const.tile([H, H], bf16, tag="w2t")
        onehot = const.tile([P, 2 * P - 1], bf16, tag="onehot")
        iota4 = const.tile([P, NSLOT], f32, tag="iota4")
        ss_all = const.tile([P, n_blocks], f32, tag="ss_all")
        ridx = const.tile([P, S // 16], i16, tag="ridx")
        meta = const.tile([1, n_chunks * NSLOT], u32, tag="meta")
        nc.sync.dma_start(out=w1t[:], in_=w1t_d[:])
        nc.sync.dma_start(out=w2t[:], in_=w2t_d[:])
        nc.sync.dma_start(out=iota4[:], in_=iota_d[:])
        nc.sync.dma_start(out=ridx[:], in_=ridx_d[:])
        nc.sync.dma_start(out=meta[:], in_=meta_d[:])
        nc.vector.memset(onehot[:], 0.0)
        nc.vector.memset(onehot[:, P - 1:P], 1.0)

        # --- phase 1: MLP table (unnormalized) + sumsq; SBUF + DRAM copies
        n0 = 0
        st = 0
        while n0 < n_pad:
            w = min(ST_W, n_pad - n0)
            nb = w // H
            xt = p1.tile([P, ST_W], bf16, tag="xt", name="xt")[:, :w]
            nc.sync.dma_start(out=xt, in_=embT[:, n0:n0 + w])
            ph1 = psA.tile([P, ST_W], f32, tag="a", name="ph1")[:, :w]
            nc.tensor.matmul(ph1, lhsT=w1t[:], rhs=xt, start=True, stop=True)
            # elu(x) = max(exp(min(x, 0)) - 1, x); exp(min(x,0)) = exp(-relu(-x))
            u_t = p1.tile([P, ST_W], bf16, tag="u", name="u")[:, :w]
            nc.scalar.activation(u_t, ph1, Act.Relu, scale=-1.0)
            e_t = p1.tile([P, ST_W], bf16, tag="e", name="e")[:, :w]
            nc.scalar.activation(e_t, u_t, Act.Exp, scale=-1.0)
            h1_t = p1.tile([P, ST_W], bf16, tag="h1", name="h1")[:, :w]
            nc.vector.scalar_tensor_tensor(
                h1_t, in0=e_t, scalar=-1.0, in1=ph1,
                op0=Alu.add, op1=Alu.max)
            pg = psB.tile([P, ST_W], f32, tag="b", name="pg")[:, :w]
            for b in range(nb):
                nc.tensor.matmul(pg[:, b * H:(b + 1) * H],
                                 lhsT=h1_t[:, b * H:(b + 1) * H],
                                 rhs=w2t[:], start=True, stop=True)
            nc.scalar.activation(table[:, n0:n0 + w], pg, Act.Copy)
            sq_t = p1.tile([P, ST_W], bf16, tag="sq", name="sq")[:, :w]
            for b in range(nb):
                nc.vector.scalar_tensor_tensor(
                    sq_t[:, b * H:(b + 1) * H],
                    in0=pg[:, b * H:(b + 1) * H], scalar=0.0,
                    in1=table[:, n0 + b * H:n0 + (b + 1) * H],
                    op0=Alu.add, op1=Alu.mult,
                    accum_out=ss_all[:, st * (ST_W // H) + b:
                                     st * (ST_W // H) + b + 1])
            # node-major stripe -> DRAM table for dynamic block loads
            nc.sync.dma_start(
                out=tbl_d[n0:n0 + w, :].rearrange("(s p) f -> p s f", p=P),
                in_=table[:, n0:n0 + w].rearrange("p (s f) -> p s f", f=H))
            n0 += w
            st += 1
        nc.sync.dma_start(out=ssq_d[:], in_=ss_all[:])

        # --- phase 2 ---
        import os as _os2
        _p1only = _os2.environ.get("KV2_PHASE1_ONLY") == "1"
        _nogather = _os2.environ.get("KV2_NO_GATHER") == "1"
        halves = (table[:, :half], table[:, half:n_pad])

        f2_tiles = {}
        pout = None
        for c in range([n_chunks, 0][_p1only]):
            if c % CPB == 0:
                bt = c // CPB
                f2t = pf2.tile([P, BATCH], bf16, tag="f2", name=f"f2_{bt}")
                f2_tiles[bt] = f2t
                if _nogather:
                    nc.vector.memset(f2t[:], 0.0)
                for (pos0, n_reg, hf) in ([] if _nogather else g_by_batch[bt]):
                    f2g = f2t[:, 0:BATCH].rearrange("p (a t) -> p a t", a=1)
                    nc.gpsimd.dma_gather(
                        f2g, halves[hf],
                        ridx[:, pos0 // 16:(pos0 + BATCH) // 16],
                        BATCH, n_reg, H,
                        transpose=True, sbuf_tokens_per_rank=P,
                        sbuf_free_dim_per_rank=256, single_packet=False,
                        queue_num=0)

            codes_t = pcode.tile([P, CHUNK], fp16, tag="codes", name=f"cd{c}")
            nc.sync.dma_start(out=codes_t[:],
                              in_=codes_d[:, c * CHUNK:(c + 1) * CHUNK])
            import os as _os
            _static_slots = _os.environ.get("KV2_STATIC_SLOTS") == "1"
            f1p = psA.tile([P, ST_W], f32, tag="a", name=f"f1_{c}")[:, :CHUNK]
            for s in range(NSLOT):
                slot = pslot.tile([P, H], bf16, tag=f"slot{s}",
                                  name=f"sl{c}_{s}")
                if _static_slots:
                    nc.scalar.dma_start(out=slot[:], in_=tbl_d[0:P, :])
                else:
                    reg = nc.scalar.alloc_register(f"roff_{c}_{s}")
                    nc.scalar.reg_load(reg, meta[0:1, c * NSLOT + s:
                                                 c * NSLOT + s + 1])
                    off = nc.scalar.snap(reg, donate=True, min_val=0,
                                         max_val=n_pad - P)
                    nc.scalar.dma_start(out=slot[:],
                                        in_=tbl_d[bass.ds(off, P), :])
                oh = poh.tile([P, CHUNK], bf16, tag=f"oh{s}",
                              name=f"oh{c}_{s}")
                nc.vector.tensor_scalar(out=oh[:], in0=codes_t[:],
                                        scalar1=iota4[:, s:s + 1],
                                        scalar2=None, op0=Alu.is_equal)
                nc.tensor.matmul(f1p[:], lhsT=slot[:], rhs=oh[:],
                                 start=(s == 0), stop=(s == NSLOT - 1))
            bt, off_b = divmod(c * CHUNK, BATCH)
            prod = pprod.tile([P, CHUNK], bf16, tag="prod", name=f"pr{c}")
            nc.vector.tensor_tensor(out=prod[:], in0=f1p[:],
                                    in1=f2_tiles[bt][:, off_b:off_b + CHUNK],
                                    op=Alu.mult)
            g, p = divmod(c, P)
            if p == 0:
                pout = psB.tile([P, ST_W], f32, tag="b",
                                name=f"po{g}")[:, :CHUNK]
            last = c == n_chunks - 1
            nc.tensor.matmul(pout[:], lhsT=onehot[:, P - 1 - p:2 * P - 1 - p],
                             rhs=prod[:], start=(p == 0),
                             stop=(p == P - 1 or last))
            if p == P - 1 or last:
                rows = p + 1
                ost = p1.tile([P, CHUNK], f32, tag="ost",
                              name=f"ost{g}")[:rows]
                nc.vector.tensor_copy(out=ost, in_=pout[:rows])
                nc.sync.dma_start(out=out_d[g, :rows], in_=ost)

    nc.compile()
    return nc


# --------------------------------------------------------------------------
# numpy emulation of phase 2 (host-side self-test)
# --------------------------------------------------------------------------

def _emulate_core(table_f32, codes, ridx, meta, half_starts, n_chunks):
    """table_f32: [n_pad, H] (bf16 values as f32). half_starts[chunk] gives
    the table base (0 or HALF) for the chunk's gather indices."""
    S = n_chunks * CHUNK
    iota4 = np.arange(P)[:, None] + 128 * np.arange(NSLOT)[None, :]
    dots = np.zeros(S, dtype=np.float32)
    for c in range(n_chunks):
        cd = codes[c * CHUNK:(c + 1) * CHUNK]
        f1 = np.zeros((H, CHUNK), dtype=np.float32)
        for s in range(NSLOT):
            oh = (cd[None, :] == iota4[:, s:s + 1]).astype(np.float32)
            r0 = int(meta[c * NSLOT + s])
            f1 += table_f32[r0:r0 + P, :].T @ oh
        ri = ridx[c * CHUNK:(c + 1) * CHUNK]
        base = half_starts[c]
        idx = np.where(ri >= 0, ri + base, 0)
        f2 = table_f32[idx, :].T
        prod = (f1 * f2).astype(BF16).astype(np.float32)
        prod[:, ri < 0] = 0.0
        dots[c * CHUNK:(c + 1) * CHUNK] = prod.sum(axis=0)
    return dots


# --------------------------------------------------------------------------
# entry point
# --------------------------------------------------------------------------

def _ensure_ntff_hook():
    """Provide antenv.axon_hooks if the image lacks it (trace support only)."""
    import sys
    import types
    try:
        import antenv.axon_hooks  # noqa: F401
        return
    except ImportError:
        pass
    try:
        import antenv
        from trn_agent_boot.trn_boot import _ntff_profile_via_ctypes
        mod = types.ModuleType("antenv.axon_hooks")
        mod._hook = _ntff_profile_via_ctypes("/opt/axon/libaxon_pjrt.so")
        mod.get_axon_ntff_profile_hook = lambda: mod._hook
        mod.set_axon_ntff_profile_hook = lambda h: setattr(mod, "_hook", h)
        sys.modules["antenv.axon_hooks"] = mod
        antenv.axon_hooks = mod
    except Exception:
        pass


def _host_prep(col, row, E):
    """Shared host prep. Returns (cores, cap, S, n_chunks, gathers)."""
    ec = E // NCORES
    cores = [_prep_core(col[k * ec:(k + 1) * ec], row[k * ec:(k + 1) * ec])
             for k in range(NCORES)]
    cap = {}
    for name in ("lo", "hi"):
        mx = max(len(cr[name][0]) for cr in cores)
        cap[name] = max(BATCH, ((mx + BATCH - 1) // BATCH) * BATCH)
    S = cap["lo"] + cap["hi"]
    n_chunks = S // CHUNK
    # shared (across cores) real-index count per gather instruction
    real = {nm: [len(cr[nm][0]) for cr in cores] for nm in ("lo", "hi")}
    gathers = []
    for pos0 in range(0, cap["lo"], BATCH):
        mx = max(min(L - pos0, BATCH) for L in real["lo"])
        n_reg = min(BATCH, ((max(mx, 0) + 15) // 16) * 16)
        if n_reg > 0:
            gathers.append((pos0, n_reg, 0))
    for pos0 in range(cap["lo"], S, BATCH):
        q0 = pos0 - cap["lo"]
        mx = max(min(L - q0, BATCH) for L in real["hi"])
        n_reg = min(BATCH, ((max(mx, 0) + 15) // 16) * 16)
        if n_reg > 0:
            gathers.append((pos0, n_reg, 1))
    return cores, cap, S, n_chunks, gathers


def kernel(emb, edge_index, W1, b1, W2, b2):
    global LAST_RESULTS
    from concourse.bass_utils import run_bass_kernel_spmd
    _ensure_ntff_hook()

    emb = np.asarray(emb, dtype=np.float32)
    W1 = np.asarray(W1, dtype=np.float32)
    W2 = np.asarray(W2, dtype=np.float32)
    b1 = np.asarray(b1, dtype=np.float32)
    b2 = np.asarray(b2, dtype=np.float32)
    assert np.abs(b1).max() == 0 and np.abs(b2).max() == 0, \
        "nonzero biases not implemented"
    col = np.asarray(edge_index[0]).astype(np.int64)
    row = np.asarray(edge_index[1]).astype(np.int64)

    n, h = emb.shape
    assert h == H
    E = col.shape[0]
    ec = E // NCORES
    n_pad = ((n + P - 1) // P) * P

    cores, cap, S, n_chunks, gathers = _host_prep(col, row, E)

    key = (n_pad, n_chunks, tuple(gathers))
    if key not in _PROG_CACHE:
        _PROG_CACHE[key] = _build_program(n_pad, n_chunks, gathers)
    nc = _PROG_CACHE[key]

    embT = np.zeros((P, n_pad), dtype=BF16)
    embT[:, :n] = emb.T.astype(BF16)
    w1t = W1.T.astype(BF16)
    w2t = W2.T.astype(BF16)
    iota4 = (np.arange(P)[:, None] + 128 * np.arange(NSLOT)[None, :]
             ).astype(np.float32)

    shared = {"lo": {}, "hi": {}}
    for (pos0, n_reg, hf) in gathers:
        nm = "lo" if hf == 0 else "hi"
        b0 = pos0 if hf == 0 else pos0 - cap["lo"]
        shared[nm][b0] = n_reg
    in_maps = []
    opos_all = []
    for cr in cores:
        lc, lr, lm, lp = _pad_region(*cr["lo"], cap["lo"], shared["lo"])
        hc, hr, hm, hp = _pad_region(*cr["hi"], cap["hi"], shared["hi"])
        codes = np.concatenate([lc, hc])
        ridx = np.concatenate([lr, hr])
        meta = np.concatenate([lm, hm])
        opos = np.concatenate([lp, hp])
        in_maps.append({
            "embT": embT, "w1t": w1t, "w2t": w2t, "iota4": iota4,
            "codes": np.broadcast_to(codes.astype(F16), (P, S)).copy(),
            "ridx": _wrap_idx(ridx),
            "meta": meta.reshape(1, -1).astype(np.uint32),
        })
        opos_all.append(opos)

    res = run_bass_kernel_spmd(nc, in_maps, core_ids=list(range(NCORES)))
    LAST_RESULTS = res

    # ---- host post: normalize + unpermute ----
    out = np.empty(E, dtype=np.float32)
    for k in range(NCORES):
        r = res.results[k]
        ssq = np.asarray(r["ssq"], dtype=np.float32)
        nrm = np.maximum(np.sqrt(ssq.T.reshape(-1)[:n]), 1e-8)
        dots = np.asarray(r["out"], dtype=np.float32).reshape(-1)[:S]
        opos = opos_all[k]
        valid = opos >= 0
        seg = out[k * ec:(k + 1) * ec]
        seg[opos[valid]] = dots[valid]
        cseg = col[k * ec:(k + 1) * ec]
        rseg = row[k * ec:(k + 1) * ec]
        seg /= nrm[cseg] * nrm[rseg]
    return out


# revision 10
# speedup vs baseline: 1.1958x; 1.1958x over previous
"""Trainium2 Bass kernel for nn_ExtractorMLP (GNN edge cosine-similarity).

Math:  out[e] = cos_sim(mlp(emb[col[e]]), mlp(emb[row[e]]))
where  mlp(x) = elu(x @ W1.T + b1) @ W2.T + b2   (b1 = b2 = 0 here)

v2 strategy (edge-data-parallel across 8 cores, one SPMD program):
  * Phase 1 (per node, replicated): UNnormalized MLP table t[v] (bf16) in
    node-major layout ([v%128 partition, v//128 stripe of 128 feats]),
    kept in SBUF (row-side dma_gather source) and mirrored to DRAM
    (col-side dynamic block loads).  Per-node sum-of-squares goes to the
    host, which applies the cosine normalization at the end.
  * Phase 2 (edges, sharded): per core, edges are bucketed by row-half
    (int16 gather-index limit), col-sorted within each bucket, and cut
    into 384-edge chunks.
      - col side (NO gather): each chunk touches <=4 distinct 128-node
        table blocks.  Block row-offsets ship as data and are loaded with
        dynamic-offset DMAs (reg_load + snap + bass.ds); one-hots
        [128, 384] are built on DVE with is_equal against host codes
        (128*slot + col%128) vs iota columns, and
        f1 = sum_s block_s.T @ onehot_s via accumulating matmuls.
      - row side: SWDGE dma_gather (transpose, SBUF source), 7680 edges
        per instruction; -1 index tails are trimmed by the ucode at zero
        descriptor-gen cost.
      - dot: DVE multiply f1(PSUM) * f2(SBUF), then contraction over the
        128 feature partitions with the sliding one-hot matmul trick,
        128 chunks per PSUM output group.
  * Host: out[e] = dots[e] / (n[col] * n[row]), n = max(sqrt(ssq), eps).
"""

import math

import numpy as np
import ml_dtypes

BF16 = ml_dtypes.bfloat16
F16 = np.float16

H = 128            # feature dim
P = 128            # partitions
CHUNK = 384        # edges per expansion chunk
NSLOT = 4          # table blocks per chunk
BATCH = 7680       # edges per dma_gather instruction / f2 tile (20 chunks)
CPB = BATCH // CHUNK
HALF = 32768       # int16 index limit: row-bucket split
NCORES = 8
ST_W = 512         # phase-1 supertile width (nodes)
PAD_CODE = 1000.0  # code value matching no slot

_PROG_CACHE: dict = {}
LAST_RESULTS = None


# --------------------------------------------------------------------------
# host-side packing
# --------------------------------------------------------------------------

def _wrap_idx(idx):
    """[S*16] int16 -> [128, S] wrapped layout (16 partitions, replicated 8x)."""
    w = idx.reshape(-1, 16).T.astype(np.int16)
    return np.tile(w, (8, 1))


def _pack_region(cols, rows, row_off):
    """Pack one row-half bucket (already col-sorted) into CHUNK streams.

    Returns (codes f32[S], ridx int64[S], meta u32[nck*NSLOT], pos int64[S]);
    pos[i] indexes into cols/rows for stream slot i (-1 = pad).  Chunk-
    internal pad gather idxs are 0 (safe valid index; f1=0 kills them),
    and converted to -1 only in the region tail by _pad_region.
    """
    n = len(cols)
    codes, ridx, meta, pos = [], [], [], []
    i = 0
    while i < n:
        blocks = {}
        c_codes = np.full(CHUNK, PAD_CODE, dtype=np.float32)
        c_ridx = np.zeros(CHUNK, dtype=np.int64)
        c_pos = np.full(CHUNK, -1, dtype=np.int64)
        j = 0
        while j < CHUNK and i < n:
            b = int(cols[i]) >> 7
            if b not in blocks:
                if len(blocks) == NSLOT:
                    break          # close chunk early (rare)
                blocks[b] = len(blocks)
            s = blocks[b]
            c_codes[j] = 128 * s + (int(cols[i]) & 127)
            c_ridx[j] = rows[i] - row_off
            c_pos[j] = i
            i += 1
            j += 1
        m = np.zeros(NSLOT, dtype=np.uint32)
        for b, s in blocks.items():
            m[s] = b * 128          # row offset into DRAM table
        codes.append(c_codes)
        ridx.append(c_ridx)
        meta.append(m)
        pos.append(c_pos)
    if not codes:
        return (np.zeros(0, np.float32), np.zeros(0, np.int64),
                np.zeros(0, np.uint32), np.zeros(0, np.int64))
    return (np.concatenate(codes), np.concatenate(ridx),
            np.concatenate(meta), np.concatenate(pos))


def _prep_core(col, row):
    """Bucket by row-half, col-sort, pack chunks."""
    lo_sel = row < HALF
    out = {}
    for name, sel, roff in (("lo", lo_sel, 0), ("hi", ~lo_sel, HALF)):
        c, r = col[sel], row[sel]
        orig = np.nonzero(sel)[0]
        order = np.argsort(c, kind="stable")
        c, r, orig = c[order], r[order], orig[order]
        codes, ridx, meta, pos = _pack_region(c, r, roff)
        opos = np.where(pos >= 0, orig[np.clip(pos, 0, None)], -1)
        out[name] = (codes, ridx, meta, opos)
    return out


def _pad_region(codes, ridx, meta, opos, cap, shared_real=None):
    """Pad a region's streams to cap edges (cap % CHUNK == 0).

    Gather idxs in [len, shared_real_of_batch) are 0 (safe pads shared
    across cores so every core trims at the same point); -1 beyond
    (trimmed by the gather ucode at zero descriptor cost)."""
    s = len(codes)
    assert s <= cap and cap % CHUNK == 0
    pc = np.full(cap, PAD_CODE, dtype=np.float32)
    pr = np.full(cap, -1, dtype=np.int64)
    po = np.full(cap, -1, dtype=np.int64)
    pm = np.zeros(cap // CHUNK * NSLOT, dtype=np.uint32)
    pc[:s] = codes
    pr[:s] = ridx
    po[:s] = opos
    pm[:len(meta)] = meta
    if shared_real is not None:
        for b0 in range(0, cap, BATCH):
            cnt = shared_real.get(b0, 0)
            lo = max(s - b0, 0)
            if cnt > lo:
                pr[b0 + lo:b0 + cnt] = 0
    return pc, pr, pm, po


# --------------------------------------------------------------------------
# device program
# --------------------------------------------------------------------------

def _build_program(n_pad, n_chunks, gathers, trace_label=""):
    """Build the shared SPMD program.

    gathers: list of (stream_pos, n_idx, half_id) gather instructions;
             each lies within one BATCH-aligned f2 tile.
    """
    import concourse.bacc as bacc
    import concourse.bass as bass
    import concourse.mybir as mybir
    import concourse.tile as tile
    from concourse import library_config
    from contextlib import ExitStack

    f32 = mybir.dt.float32
    bf16 = mybir.dt.bfloat16
    fp16 = mybir.dt.float16
    i16 = mybir.dt.int16
    u32 = mybir.dt.uint32
    Alu = mybir.AluOpType
    Act = mybir.ActivationFunctionType

    S = n_chunks * CHUNK
    n_groups = math.ceil(n_chunks / P)
    n_blocks = n_pad // H
    half = min(HALF, n_pad)
    n_batches = S // BATCH

    nc = bacc.Bacc("TRN2", target_bir_lowering=False, debug=False,
                   num_devices=NCORES, num_swdge_queues=1)

    embT = nc.dram_tensor("embT", [P, n_pad], bf16, kind="ExternalInput")
    w1t_d = nc.dram_tensor("w1t", [H, H], bf16, kind="ExternalInput")
    w2t_d = nc.dram_tensor("w2t", [H, H], bf16, kind="ExternalInput")
    oh_d = nc.dram_tensor("oh", [P, n_chunks * NSLOT * CHUNK], bf16,
                          kind="ExternalInput")
    ridx_d = nc.dram_tensor("ridx", [P, S // 16], i16, kind="ExternalInput")
    meta_d = nc.dram_tensor("meta", [1, n_chunks * NSLOT], u32,
                            kind="ExternalInput")
    tbl_d = nc.dram_tensor("tbl_dram", [n_pad, H], bf16, kind="ExternalOutput")
    ssq_d = nc.dram_tensor("ssq", [P, n_blocks], f32, kind="ExternalOutput")
    out_d = nc.dram_tensor("out", [n_groups, P, CHUNK], f32,
                           kind="ExternalOutput")

    # group gathers by f2 batch tile
    g_by_batch = [[] for _ in range(n_batches)]
    for (pos0, n_reg, hf) in gathers:
        bt = pos0 // BATCH
        assert pos0 % BATCH == 0 and 0 < n_reg <= BATCH
        g_by_batch[bt].append((pos0, n_reg, hf))

    with ExitStack() as ctx:
        tc = ctx.enter_context(tile.TileContext(nc))
        const = ctx.enter_context(tc.tile_pool(name="const", bufs=1))
        p1 = ctx.enter_context(tc.tile_pool(name="p1", bufs=3))
        pohd = ctx.enter_context(tc.tile_pool(name="pohd", bufs=4))
        pridx = ctx.enter_context(tc.tile_pool(name="pridx", bufs=2))
        pslot = ctx.enter_context(tc.tile_pool(name="pslot", bufs=2 * NSLOT))
        pf2 = ctx.enter_context(tc.tile_pool(name="pf2", bufs=2))
        pprod = ctx.enter_context(tc.tile_pool(name="pprod", bufs=4))
        psA = ctx.enter_context(tc.tile_pool(name="psA", bufs=4, space="PSUM"))
        psB = ctx.enter_context(tc.tile_pool(name="psB", bufs=4, space="PSUM"))

        nc.gpsimd.load_library(library_config.mlp)

        # --- constants / persistent tiles ---
        table = const.tile([P, n_pad], bf16, tag="table")
        w1t = const.tile([H, H], bf16, tag="w1t")
        w2t = const.tile([H, H], bf16, tag="w2t")
        onehot = const.tile([P, 2 * P - 1], bf16, tag="onehot")
        ss_all = const.tile([P, n_blocks], f32, tag="ss_all")
        meta = const.tile([1, n_chunks * NSLOT], u32, tag="meta")
        nc.sync.dma_start(out=w1t[:], in_=w1t_d[:])
        nc.sync.dma_start(out=w2t[:], in_=w2t_d[:])
        nc.sync.dma_start(out=meta[:], in_=meta_d[:])
        nc.vector.memset(onehot[:], 0.0)
        nc.vector.memset(onehot[:, P - 1:P], 1.0)

        # --- phase 1: MLP table (unnormalized) + sumsq; SBUF + DRAM copies
        n0 = 0
        st = 0
        while n0 < n_pad:
            w = min(ST_W, n_pad - n0)
            nb = w // H
            xt = p1.tile([P, ST_W], bf16, tag="xt", name="xt")[:, :w]
            nc.sync.dma_start(out=xt, in_=embT[:, n0:n0 + w])
            ph1 = psA.tile([P, ST_W], f32, tag="a", name="ph1")[:, :w]
            nc.tensor.matmul(ph1, lhsT=w1t[:], rhs=xt, start=True, stop=True)
            # elu(x) = max(exp(min(x, 0)) - 1, x); exp(min(x,0)) = exp(-relu(-x))
            u_t = p1.tile([P, ST_W], bf16, tag="u", name="u")[:, :w]
            nc.scalar.activation(u_t, ph1, Act.Relu, scale=-1.0)
            e_t = p1.tile([P, ST_W], bf16, tag="e", name="e")[:, :w]
            nc.scalar.activation(e_t, u_t, Act.Exp, scale=-1.0)
            h1_t = p1.tile([P, ST_W], bf16, tag="h1", name="h1")[:, :w]
            nc.vector.scalar_tensor_tensor(
                h1_t, in0=e_t, scalar=-1.0, in1=ph1,
                op0=Alu.add, op1=Alu.max)
            pg = psB.tile([P, ST_W], f32, tag="b", name="pg")[:, :w]
            for b in range(nb):
                nc.tensor.matmul(pg[:, b * H:(b + 1) * H],
                                 lhsT=h1_t[:, b * H:(b + 1) * H],
                                 rhs=w2t[:], start=True, stop=True)
            nc.scalar.activation(table[:, n0:n0 + w], pg, Act.Copy)
            sq_t = p1.tile([P, ST_W], bf16, tag="sq", name="sq")[:, :w]
            for b in range(nb):
                nc.vector.scalar_tensor_tensor(
                    sq_t[:, b * H:(b + 1) * H],
                    in0=pg[:, b * H:(b + 1) * H], scalar=0.0,
                    in1=table[:, n0 + b * H:n0 + (b + 1) * H],
                    op0=Alu.add, op1=Alu.mult,
                    accum_out=ss_all[:, st * (ST_W // H) + b:
                                     st * (ST_W // H) + b + 1])
            # node-major stripe -> DRAM table for dynamic block loads
            nc.sync.dma_start(
                out=tbl_d[n0:n0 + w, :].rearrange("(s p) f -> p s f", p=P),
                in_=table[:, n0:n0 + w].rearrange("p (s f) -> p s f", f=H))
            n0 += w
            st += 1
        nc.sync.dma_start(out=ssq_d[:], in_=ss_all[:])

        # --- phase 2 ---
        import os as _os2
        _p1only = _os2.environ.get("KV2_PHASE1_ONLY") == "1"
        _nogather = _os2.environ.get("KV2_NO_GATHER") == "1"
        halves = (table[:, :half], table[:, half:n_pad])

        f2_tiles = {}
        pout = None
        for c in range([n_chunks, 0][_p1only]):
            if c % CPB == 0:
                bt = c // CPB
                f2t = pf2.tile([P, BATCH], bf16, tag="f2", name=f"f2_{bt}")
                f2_tiles[bt] = f2t
                if _nogather:
                    nc.vector.memset(f2t[:], 0.0)
                for (pos0, n_reg, hf) in ([] if _nogather else g_by_batch[bt]):
                    rxt = pridx.tile([P, BATCH // 16], i16, tag="rx",
                                     name=f"rx{bt}")
                    nc.sync.dma_start(
                        out=rxt[:],
                        in_=ridx_d[:, pos0 // 16:(pos0 + BATCH) // 16])
                    f2g = f2t[:, 0:BATCH].rearrange("p (a t) -> p a t", a=1)
                    nc.gpsimd.dma_gather(
                        f2g, halves[hf], rxt[:],
                        BATCH, n_reg, H,
                        transpose=True, sbuf_tokens_per_rank=P,
                        sbuf_free_dim_per_rank=256, single_packet=False,
                        queue_num=0)

            oht = pohd.tile([P, NSLOT * CHUNK], bf16, tag="ohd",
                            name=f"oh{c}")
            nc.sync.dma_start(
                out=oht[:],
                in_=oh_d[:, c * NSLOT * CHUNK:(c + 1) * NSLOT * CHUNK])
            import os as _os
            _static_slots = _os.environ.get("KV2_STATIC_SLOTS") == "1"
            f1p = psA.tile([P, ST_W], f32, tag="a", name=f"f1_{c}")[:, :CHUNK]
            for s in range(NSLOT):
                slot = pslot.tile([P, H], bf16, tag=f"slot{s}",
                                  name=f"sl{c}_{s}")
                if _static_slots:
                    nc.scalar.dma_start(out=slot[:], in_=tbl_d[0:P, :])
                else:
                    reg = nc.scalar.alloc_register(f"roff_{c}_{s}")
                    nc.scalar.reg_load(reg, meta[0:1, c * NSLOT + s:
                                                 c * NSLOT + s + 1])
                    off = nc.scalar.snap(reg, donate=True, min_val=0,
                                         max_val=n_pad - P)
                    nc.scalar.dma_start(out=slot[:],
                                        in_=tbl_d[bass.ds(off, P), :])
                nc.tensor.matmul(f1p[:], lhsT=slot[:],
                                 rhs=oht[:, s * CHUNK:(s + 1) * CHUNK],
                                 start=(s == 0), stop=(s == NSLOT - 1))
            bt, off_b = divmod(c * CHUNK, BATCH)
            prod = pprod.tile([P, CHUNK], bf16, tag="prod", name=f"pr{c}")
            nc.vector.tensor_tensor(out=prod[:], in0=f1p[:],
                                    in1=f2_tiles[bt][:, off_b:off_b + CHUNK],
                                    op=Alu.mult)
            g, p = divmod(c, P)
            if p == 0:
                pout = psB.tile([P, ST_W], f32, tag="b",
                                name=f"po{g}")[:, :CHUNK]
            last = c == n_chunks - 1
            nc.tensor.matmul(pout[:], lhsT=onehot[:, P - 1 - p:2 * P - 1 - p],
                             rhs=prod[:], start=(p == 0),
                             stop=(p == P - 1 or last))
            if p == P - 1 or last:
                rows = p + 1
                ost = p1.tile([P, CHUNK], f32, tag="ost",
                              name=f"ost{g}")[:rows]
                nc.vector.tensor_copy(out=ost, in_=pout[:rows])
                nc.sync.dma_start(out=out_d[g, :rows], in_=ost)

    nc.compile()
    return nc


# --------------------------------------------------------------------------
# numpy emulation of phase 2 (host-side self-test)
# --------------------------------------------------------------------------

def _emulate_core(table_f32, codes, ridx, meta, half_starts, n_chunks):
    """table_f32: [n_pad, H] (bf16 values as f32). half_starts[chunk] gives
    the table base (0 or HALF) for the chunk's gather indices."""
    S = n_chunks * CHUNK
    iota4 = np.arange(P)[:, None] + 128 * np.arange(NSLOT)[None, :]
    dots = np.zeros(S, dtype=np.float32)
    for c in range(n_chunks):
        cd = codes[c * CHUNK:(c + 1) * CHUNK]
        f1 = np.zeros((H, CHUNK), dtype=np.float32)
        for s in range(NSLOT):
            oh = (cd[None, :] == iota4[:, s:s + 1]).astype(np.float32)
            r0 = int(meta[c * NSLOT + s])
            f1 += table_f32[r0:r0 + P, :].T @ oh
        ri = ridx[c * CHUNK:(c + 1) * CHUNK]
        base = half_starts[c]
        idx = np.where(ri >= 0, ri + base, 0)
        f2 = table_f32[idx, :].T
        prod = (f1 * f2).astype(BF16).astype(np.float32)
        prod[:, ri < 0] = 0.0
        dots[c * CHUNK:(c + 1) * CHUNK] = prod.sum(axis=0)
    return dots


# --------------------------------------------------------------------------
# entry point
# --------------------------------------------------------------------------

def _ensure_ntff_hook():
    """Provide antenv.axon_hooks if the image lacks it (trace support only)."""
    import sys
    import types
    try:
        import antenv.axon_hooks  # noqa: F401
        return
    except ImportError:
        pass
    try:
        import antenv
        from trn_agent_boot.trn_boot import _ntff_profile_via_ctypes
        mod = types.ModuleType("antenv.axon_hooks")
        mod._hook = _ntff_profile_via_ctypes("/opt/axon/libaxon_pjrt.so")
        mod.get_axon_ntff_profile_hook = lambda: mod._hook
        mod.set_axon_ntff_profile_hook = lambda h: setattr(mod, "_hook", h)
        sys.modules["antenv.axon_hooks"] = mod
        antenv.axon_hooks = mod
    except Exception:
        pass


def _host_prep(col, row, E):
    """Shared host prep. Returns (cores, cap, S, n_chunks, gathers)."""
    ec = E // NCORES
    cores = [_prep_core(col[k * ec:(k + 1) * ec], row[k * ec:(k + 1) * ec])
             for k in range(NCORES)]
    cap = {}
    for name in ("lo", "hi"):
        mx = max(len(cr[name][0]) for cr in cores)
        cap[name] = max(BATCH, ((mx + BATCH - 1) // BATCH) * BATCH)
    S = cap["lo"] + cap["hi"]
    n_chunks = S // CHUNK
    # shared (across cores) real-index count per gather instruction
    real = {nm: [len(cr[nm][0]) for cr in cores] for nm in ("lo", "hi")}
    gathers = []
    for pos0 in range(0, cap["lo"], BATCH):
        mx = max(min(L - pos0, BATCH) for L in real["lo"])
        n_reg = min(BATCH, ((max(mx, 0) + 15) // 16) * 16)
        if n_reg > 0:
            gathers.append((pos0, n_reg, 0))
    for pos0 in range(cap["lo"], S, BATCH):
        q0 = pos0 - cap["lo"]
        mx = max(min(L - q0, BATCH) for L in real["hi"])
        n_reg = min(BATCH, ((max(mx, 0) + 15) // 16) * 16)
        if n_reg > 0:
            gathers.append((pos0, n_reg, 1))
    return cores, cap, S, n_chunks, gathers


def kernel(emb, edge_index, W1, b1, W2, b2):
    global LAST_RESULTS
    from concourse.bass_utils import run_bass_kernel_spmd
    _ensure_ntff_hook()

    emb = np.asarray(emb, dtype=np.float32)
    W1 = np.asarray(W1, dtype=np.float32)
    W2 = np.asarray(W2, dtype=np.float32)
    b1 = np.asarray(b1, dtype=np.float32)
    b2 = np.asarray(b2, dtype=np.float32)
    assert np.abs(b1).max() == 0 and np.abs(b2).max() == 0, \
        "nonzero biases not implemented"
    col = np.asarray(edge_index[0]).astype(np.int64)
    row = np.asarray(edge_index[1]).astype(np.int64)

    n, h = emb.shape
    assert h == H
    E = col.shape[0]
    ec = E // NCORES
    n_pad = ((n + P - 1) // P) * P

    cores, cap, S, n_chunks, gathers = _host_prep(col, row, E)

    key = (n_pad, n_chunks, tuple(gathers))
    if key not in _PROG_CACHE:
        _PROG_CACHE[key] = _build_program(n_pad, n_chunks, gathers)
    nc = _PROG_CACHE[key]

    embT = np.zeros((P, n_pad), dtype=BF16)
    embT[:, :n] = emb.T.astype(BF16)
    w1t = W1.T.astype(BF16)
    w2t = W2.T.astype(BF16)

    shared = {"lo": {}, "hi": {}}
    for (pos0, n_reg, hf) in gathers:
        nm = "lo" if hf == 0 else "hi"
        b0 = pos0 if hf == 0 else pos0 - cap["lo"]
        shared[nm][b0] = n_reg
    in_maps = []
    opos_all = []
    for cr in cores:
        lc, lr, lm, lp = _pad_region(*cr["lo"], cap["lo"], shared["lo"])
        hc, hr, hm, hp = _pad_region(*cr["hi"], cap["hi"], shared["hi"])
        codes = np.concatenate([lc, hc])
        ridx = np.concatenate([lr, hr])
        meta = np.concatenate([lm, hm])
        opos = np.concatenate([lp, hp])
        # onehot blocks: oh[d, (c, s, e)] = 1 iff codes[c*CHUNK+e] == 128*s+d
        cd = codes.reshape(-1, CHUNK).astype(np.int32)      # [n_chunks, CHUNK]
        ohs = np.zeros((P, n_chunks, NSLOT, CHUNK), dtype=BF16)
        ccp, eep = np.meshgrid(np.arange(n_chunks), np.arange(CHUNK),
                               indexing="ij")
        vv = cd < 128 * NSLOT
        ohs[cd[vv] & 127, ccp[vv], cd[vv] >> 7, eep[vv]] = 1
        in_maps.append({
            "embT": embT, "w1t": w1t, "w2t": w2t,
            "oh": ohs.reshape(P, -1),
            "ridx": _wrap_idx(ridx),
            "meta": meta.reshape(1, -1).astype(np.uint32),
        })
        opos_all.append(opos)

    res = run_bass_kernel_spmd(nc, in_maps, core_ids=list(range(NCORES)))
    LAST_RESULTS = res

    # ---- host post: normalize + unpermute ----
    out = np.empty(E, dtype=np.float32)
    for k in range(NCORES):
        r = res.results[k]
        ssq = np.asarray(r["ssq"], dtype=np.float32)
        nrm = np.maximum(np.sqrt(ssq.T.reshape(-1)[:n]), 1e-8)
        dots = np.asarray(r["out"], dtype=np.float32).reshape(-1)[:S]
        opos = opos_all[k]
        valid = opos >= 0
        seg = out[k * ec:(k + 1) * ec]
        seg[opos[valid]] = dots[valid]
        cseg = col[k * ec:(k + 1) * ec]
        rseg = row[k * ec:(k + 1) * ec]
        seg /= nrm[cseg] * nrm[rseg]
    return out


# revision 11
# speedup vs baseline: 1.4274x; 1.1936x over previous
"""Trainium2 Bass kernel for nn_ExtractorMLP (GNN edge cosine-similarity).

Math:  out[e] = cos_sim(mlp(emb[col[e]]), mlp(emb[row[e]]))
where  mlp(x) = elu(x @ W1.T + b1) @ W2.T + b2   (b1 = b2 = 0 here)

v2 strategy (edge-data-parallel across 8 cores, one SPMD program):
  * Phase 1 (per node, replicated): UNnormalized MLP table t[v] (bf16) in
    node-major layout ([v%128 partition, v//128 stripe of 128 feats]),
    kept in SBUF (row-side dma_gather source) and mirrored to DRAM
    (col-side dynamic block loads).  Per-node sum-of-squares goes to the
    host, which applies the cosine normalization at the end.
  * Phase 2 (edges, sharded): per core, edges are bucketed by row-half
    (int16 gather-index limit), col-sorted within each bucket, and cut
    into 384-edge chunks.
      - col side (NO gather): each chunk touches <=4 distinct 128-node
        table blocks.  Block row-offsets ship as data and are loaded with
        dynamic-offset DMAs (reg_load + snap + bass.ds); one-hots
        [128, 384] are built on DVE with is_equal against host codes
        (128*slot + col%128) vs iota columns, and
        f1 = sum_s block_s.T @ onehot_s via accumulating matmuls.
      - row side: SWDGE dma_gather (transpose, SBUF source), 7680 edges
        per instruction; -1 index tails are trimmed by the ucode at zero
        descriptor-gen cost.
      - dot: DVE multiply f1(PSUM) * f2(SBUF), then contraction over the
        128 feature partitions with the sliding one-hot matmul trick,
        128 chunks per PSUM output group.
  * Host: out[e] = dots[e] / (n[col] * n[row]), n = max(sqrt(ssq), eps).
"""

import math

import numpy as np
import ml_dtypes

BF16 = ml_dtypes.bfloat16
F16 = np.float16

H = 128            # feature dim
P = 128            # partitions
CHUNK = 384        # edges per expansion chunk
NSLOT = 4          # table blocks per chunk
BATCH = 7680       # edges per dma_gather instruction / f2 tile (20 chunks)
CPB = BATCH // CHUNK
HALF = 32768       # int16 index limit: row-bucket split
NCORES = 8
ST_W = 512         # phase-1 supertile width (nodes)
PAD_CODE = 1000.0  # code value matching no slot

_PROG_CACHE: dict = {}
LAST_RESULTS = None


# --------------------------------------------------------------------------
# host-side packing
# --------------------------------------------------------------------------

def _wrap_idx(idx):
    """[S*16] int16 -> [128, S] wrapped layout (16 partitions, replicated 8x)."""
    w = idx.reshape(-1, 16).T.astype(np.int16)
    return np.tile(w, (8, 1))


def _pack_region(cols, rows, row_off):
    """Pack one row-half bucket (already col-sorted) into CHUNK streams.

    Returns (codes f32[S], ridx int64[S], meta u32[nck*NSLOT], pos int64[S]);
    pos[i] indexes into cols/rows for stream slot i (-1 = pad).  Chunk-
    internal pad gather idxs are 0 (safe valid index; f1=0 kills them),
    and converted to -1 only in the region tail by _pad_region.
    """
    n = len(cols)
    codes, ridx, meta, pos = [], [], [], []
    i = 0
    while i < n:
        blocks = {}
        c_codes = np.full(CHUNK, PAD_CODE, dtype=np.float32)
        c_ridx = np.zeros(CHUNK, dtype=np.int64)
        c_pos = np.full(CHUNK, -1, dtype=np.int64)
        j = 0
        while j < CHUNK and i < n:
            b = int(cols[i]) >> 7
            if b not in blocks:
                if len(blocks) == NSLOT:
                    break          # close chunk early (rare)
                blocks[b] = len(blocks)
            s = blocks[b]
            c_codes[j] = 128 * s + (int(cols[i]) & 127)
            c_ridx[j] = rows[i] - row_off
            c_pos[j] = i
            i += 1
            j += 1
        m = np.zeros(NSLOT, dtype=np.uint32)
        for b, s in blocks.items():
            m[s] = b * 128          # row offset into DRAM table
        codes.append(c_codes)
        ridx.append(c_ridx)
        meta.append(m)
        pos.append(c_pos)
    if not codes:
        return (np.zeros(0, np.float32), np.zeros(0, np.int64),
                np.zeros(0, np.uint32), np.zeros(0, np.int64))
    return (np.concatenate(codes), np.concatenate(ridx),
            np.concatenate(meta), np.concatenate(pos))


def _prep_core(col, row):
    """Bucket by row-half, col-sort, pack chunks."""
    lo_sel = row < HALF
    out = {}
    for name, sel, roff in (("lo", lo_sel, 0), ("hi", ~lo_sel, HALF)):
        c, r = col[sel], row[sel]
        orig = np.nonzero(sel)[0]
        order = np.argsort(c, kind="stable")
        c, r, orig = c[order], r[order], orig[order]
        codes, ridx, meta, pos = _pack_region(c, r, roff)
        opos = np.where(pos >= 0, orig[np.clip(pos, 0, None)], -1)
        out[name] = (codes, ridx, meta, opos)
    return out


def _pad_region(codes, ridx, meta, opos, cap, shared_real=None):
    """Pad a region's streams to cap edges (cap % CHUNK == 0).

    Gather idxs in [len, shared_real_of_batch) are 0 (safe pads shared
    across cores so every core trims at the same point); -1 beyond
    (trimmed by the gather ucode at zero descriptor cost)."""
    s = len(codes)
    assert s <= cap and cap % CHUNK == 0
    pc = np.full(cap, PAD_CODE, dtype=np.float32)
    pr = np.full(cap, -1, dtype=np.int64)
    po = np.full(cap, -1, dtype=np.int64)
    pm = np.zeros(cap // CHUNK * NSLOT, dtype=np.uint32)
    pc[:s] = codes
    pr[:s] = ridx
    po[:s] = opos
    pm[:len(meta)] = meta
    if shared_real is not None:
        for b0 in range(0, cap, BATCH):
            cnt = shared_real.get(b0, 0)
            lo = max(s - b0, 0)
            if cnt > lo:
                pr[b0 + lo:b0 + cnt] = 0
    return pc, pr, pm, po


# --------------------------------------------------------------------------
# device program
# --------------------------------------------------------------------------

def _build_program(n_pad, n_chunks, gathers, trace_label=""):
    """Build the shared SPMD program.

    gathers: list of (stream_pos, n_idx, half_id) gather instructions;
             each lies within one BATCH-aligned f2 tile.
    """
    import concourse.bacc as bacc
    import concourse.bass as bass
    import concourse.mybir as mybir
    import concourse.tile as tile
    from concourse import library_config
    from contextlib import ExitStack

    f32 = mybir.dt.float32
    bf16 = mybir.dt.bfloat16
    fp16 = mybir.dt.float16
    i16 = mybir.dt.int16
    u32 = mybir.dt.uint32
    Alu = mybir.AluOpType
    Act = mybir.ActivationFunctionType

    S = n_chunks * CHUNK
    n_groups = math.ceil(n_chunks / P)
    n_blocks = n_pad // H
    half = min(HALF, n_pad)
    n_batches = S // BATCH

    nc = bacc.Bacc("TRN2", target_bir_lowering=False, debug=False,
                   num_devices=NCORES, num_swdge_queues=1)

    embT = nc.dram_tensor("embT", [P, n_pad], bf16, kind="ExternalInput")
    w1t_d = nc.dram_tensor("w1t", [H, H], bf16, kind="ExternalInput")
    w2t_d = nc.dram_tensor("w2t", [H, H], bf16, kind="ExternalInput")
    oh_d = nc.dram_tensor("oh", [P, n_chunks * NSLOT * CHUNK], bf16,
                          kind="ExternalInput")
    ridx_d = nc.dram_tensor("ridx", [P, S // 16], i16, kind="ExternalInput")
    meta_d = nc.dram_tensor("meta", [1, n_chunks * NSLOT], u32,
                            kind="ExternalInput")
    tbl_d = nc.dram_tensor("tbl_dram", [n_pad, H], bf16, kind="ExternalOutput")
    ssq_d = nc.dram_tensor("ssq", [P, n_blocks], f32, kind="ExternalOutput")
    out_d = nc.dram_tensor("out", [n_groups, P, CHUNK], f32,
                           kind="ExternalOutput")

    # group gathers by f2 batch tile
    g_by_batch = [[] for _ in range(n_batches)]
    for (pos0, n_reg, hf) in gathers:
        bt = pos0 // BATCH
        assert pos0 % BATCH == 0 and 0 < n_reg <= BATCH
        g_by_batch[bt].append((pos0, n_reg, hf))

    with ExitStack() as ctx:
        tc = ctx.enter_context(tile.TileContext(nc))
        const = ctx.enter_context(tc.tile_pool(name="const", bufs=1))
        p1 = ctx.enter_context(tc.tile_pool(name="p1", bufs=3))
        pohd = ctx.enter_context(tc.tile_pool(name="pohd", bufs=4))
        pridx = ctx.enter_context(tc.tile_pool(name="pridx", bufs=2))
        pslot = ctx.enter_context(tc.tile_pool(name="pslot", bufs=2 * NSLOT))
        pf2 = ctx.enter_context(tc.tile_pool(name="pf2", bufs=2))
        pprod = ctx.enter_context(tc.tile_pool(name="pprod", bufs=4))
        psA = ctx.enter_context(tc.tile_pool(name="psA", bufs=4, space="PSUM"))
        psB = ctx.enter_context(tc.tile_pool(name="psB", bufs=4, space="PSUM"))

        nc.gpsimd.load_library(library_config.mlp)

        # --- constants / persistent tiles ---
        table = const.tile([P, n_pad], bf16, tag="table")
        w1t = const.tile([H, H], bf16, tag="w1t")
        w2t = const.tile([H, H], bf16, tag="w2t")
        onehot = const.tile([P, 2 * P - 1], bf16, tag="onehot")
        ss_all = const.tile([P, n_blocks], f32, tag="ss_all")
        meta = const.tile([1, n_chunks * NSLOT], u32, tag="meta")
        nc.sync.dma_start(out=w1t[:], in_=w1t_d[:])
        nc.sync.dma_start(out=w2t[:], in_=w2t_d[:])
        nc.sync.dma_start(out=meta[:], in_=meta_d[:])
        nc.vector.memset(onehot[:], 0.0)
        nc.vector.memset(onehot[:, P - 1:P], 1.0)

        # --- phase 1: MLP table (unnormalized) + sumsq; SBUF + DRAM copies
        n0 = 0
        st = 0
        while n0 < n_pad:
            w = min(ST_W, n_pad - n0)
            nb = w // H
            xt = p1.tile([P, ST_W], bf16, tag="xt", name="xt")[:, :w]
            nc.sync.dma_start(out=xt, in_=embT[:, n0:n0 + w])
            ph1 = psA.tile([P, ST_W], f32, tag="a", name="ph1")[:, :w]
            nc.tensor.matmul(ph1, lhsT=w1t[:], rhs=xt, start=True, stop=True)
            # elu(x) = max(exp(min(x, 0)) - 1, x); exp(min(x,0)) = exp(-relu(-x))
            u_t = p1.tile([P, ST_W], bf16, tag="u", name="u")[:, :w]
            nc.scalar.activation(u_t, ph1, Act.Relu, scale=-1.0)
            e_t = p1.tile([P, ST_W], bf16, tag="e", name="e")[:, :w]
            nc.scalar.activation(e_t, u_t, Act.Exp, scale=-1.0)
            h1_t = p1.tile([P, ST_W], bf16, tag="h1", name="h1")[:, :w]
            nc.vector.scalar_tensor_tensor(
                h1_t, in0=e_t, scalar=-1.0, in1=ph1,
                op0=Alu.add, op1=Alu.max)
            pg = psB.tile([P, ST_W], f32, tag="b", name="pg")[:, :w]
            for b in range(nb):
                nc.tensor.matmul(pg[:, b * H:(b + 1) * H],
                                 lhsT=h1_t[:, b * H:(b + 1) * H],
                                 rhs=w2t[:], start=True, stop=True)
            nc.scalar.activation(table[:, n0:n0 + w], pg, Act.Copy)
            sq_t = p1.tile([P, ST_W], bf16, tag="sq", name="sq")[:, :w]
            for b in range(nb):
                nc.vector.scalar_tensor_tensor(
                    sq_t[:, b * H:(b + 1) * H],
                    in0=pg[:, b * H:(b + 1) * H], scalar=0.0,
                    in1=table[:, n0 + b * H:n0 + (b + 1) * H],
                    op0=Alu.add, op1=Alu.mult,
                    accum_out=ss_all[:, st * (ST_W // H) + b:
                                     st * (ST_W // H) + b + 1])
            # node-major stripe -> DRAM table for dynamic block loads
            nc.sync.dma_start(
                out=tbl_d[n0:n0 + w, :].rearrange("(s p) f -> p s f", p=P),
                in_=table[:, n0:n0 + w].rearrange("p (s f) -> p s f", f=H))
            n0 += w
            st += 1
        nc.sync.dma_start(out=ssq_d[:], in_=ss_all[:])

        # --- phase 2 ---
        import os as _os2
        _p1only = _os2.environ.get("KV2_PHASE1_ONLY") == "1"
        _nogather = _os2.environ.get("KV2_NO_GATHER") == "1"
        halves = (table[:, :half], table[:, half:n_pad])

        f2_tiles = {}
        pout = None
        for c in range([n_chunks, 0][_p1only]):
            if c % CPB == 0:
                bt = c // CPB
                f2t = pf2.tile([P, BATCH], bf16, tag="f2", name=f"f2_{bt}")
                f2_tiles[bt] = f2t
                if _nogather:
                    nc.vector.memset(f2t[:], 0.0)
                for (pos0, n_reg, hf) in ([] if _nogather else g_by_batch[bt]):
                    rxt = pridx.tile([P, BATCH // 16], i16, tag="rx",
                                     name=f"rx{bt}")
                    nc.sync.dma_start(
                        out=rxt[:],
                        in_=ridx_d[:, pos0 // 16:(pos0 + BATCH) // 16])
                    f2g = f2t[:, 0:BATCH].rearrange("p (a t) -> p a t", a=1)
                    nc.gpsimd.dma_gather(
                        f2g, halves[hf], rxt[:],
                        BATCH, n_reg, H,
                        transpose=True, sbuf_tokens_per_rank=P,
                        sbuf_free_dim_per_rank=256, single_packet=False,
                        queue_num=0)

            oht = pohd.tile([P, NSLOT * CHUNK], bf16, tag="ohd",
                            name=f"oh{c}")
            nc.vector.dma_start(
                out=oht[:],
                in_=oh_d[:, c * NSLOT * CHUNK:(c + 1) * NSLOT * CHUNK]) \
                if False else nc.sync.dma_start(
                out=oht[:],
                in_=oh_d[:, c * NSLOT * CHUNK:(c + 1) * NSLOT * CHUNK])
            import os as _os
            _static_slots = _os.environ.get("KV2_STATIC_SLOTS") == "1"
            f1p = psA.tile([P, ST_W], f32, tag="a", name=f"f1_{c}")[:, :CHUNK]
            for s in range(NSLOT):
                slot = pslot.tile([P, H], bf16, tag=f"slot{s}",
                                  name=f"sl{c}_{s}")
                eng = nc.scalar if s % 2 == 0 else nc.sync
                if _static_slots:
                    eng.dma_start(out=slot[:], in_=tbl_d[0:P, :])
                else:
                    reg = eng.alloc_register(f"roff_{c}_{s}")
                    reg_src = meta[0:1, c * NSLOT + s:c * NSLOT + s + 1]
                    eng.reg_load(reg, reg_src)
                    off = eng.snap(reg, donate=True, min_val=0,
                                   max_val=n_pad - P)
                    eng.dma_start(out=slot[:],
                                  in_=tbl_d[bass.ds(off, P), :])
                nc.tensor.matmul(f1p[:], lhsT=slot[:],
                                 rhs=oht[:, s * CHUNK:(s + 1) * CHUNK],
                                 start=(s == 0), stop=(s == NSLOT - 1))
            bt, off_b = divmod(c * CHUNK, BATCH)
            prod = pprod.tile([P, CHUNK], bf16, tag="prod", name=f"pr{c}")
            nc.vector.tensor_tensor(out=prod[:], in0=f1p[:],
                                    in1=f2_tiles[bt][:, off_b:off_b + CHUNK],
                                    op=Alu.mult)
            g, p = divmod(c, P)
            if p == 0:
                pout = psB.tile([P, ST_W], f32, tag="b",
                                name=f"po{g}")[:, :CHUNK]
            last = c == n_chunks - 1
            nc.tensor.matmul(pout[:], lhsT=onehot[:, P - 1 - p:2 * P - 1 - p],
                             rhs=prod[:], start=(p == 0),
                             stop=(p == P - 1 or last))
            if p == P - 1 or last:
                rows = p + 1
                ost = p1.tile([P, CHUNK], f32, tag="ost",
                              name=f"ost{g}")[:rows]
                nc.vector.tensor_copy(out=ost, in_=pout[:rows])
                nc.sync.dma_start(out=out_d[g, :rows], in_=ost)

    nc.compile()
    return nc


# --------------------------------------------------------------------------
# numpy emulation of phase 2 (host-side self-test)
# --------------------------------------------------------------------------

def _emulate_core(table_f32, codes, ridx, meta, half_starts, n_chunks):
    """table_f32: [n_pad, H] (bf16 values as f32). half_starts[chunk] gives
    the table base (0 or HALF) for the chunk's gather indices."""
    S = n_chunks * CHUNK
    iota4 = np.arange(P)[:, None] + 128 * np.arange(NSLOT)[None, :]
    dots = np.zeros(S, dtype=np.float32)
    for c in range(n_chunks):
        cd = codes[c * CHUNK:(c + 1) * CHUNK]
        f1 = np.zeros((H, CHUNK), dtype=np.float32)
        for s in range(NSLOT):
            oh = (cd[None, :] == iota4[:, s:s + 1]).astype(np.float32)
            r0 = int(meta[c * NSLOT + s])
            f1 += table_f32[r0:r0 + P, :].T @ oh
        ri = ridx[c * CHUNK:(c + 1) * CHUNK]
        base = half_starts[c]
        idx = np.where(ri >= 0, ri + base, 0)
        f2 = table_f32[idx, :].T
        prod = (f1 * f2).astype(BF16).astype(np.float32)
        prod[:, ri < 0] = 0.0
        dots[c * CHUNK:(c + 1) * CHUNK] = prod.sum(axis=0)
    return dots


# --------------------------------------------------------------------------
# entry point
# --------------------------------------------------------------------------

def _ensure_ntff_hook():
    """Provide antenv.axon_hooks if the image lacks it (trace support only)."""
    import sys
    import types
    try:
        import antenv.axon_hooks  # noqa: F401
        return
    except ImportError:
        pass
    try:
        import antenv
        from trn_agent_boot.trn_boot import _ntff_profile_via_ctypes
        mod = types.ModuleType("antenv.axon_hooks")
        mod._hook = _ntff_profile_via_ctypes("/opt/axon/libaxon_pjrt.so")
        mod.get_axon_ntff_profile_hook = lambda: mod._hook
        mod.set_axon_ntff_profile_hook = lambda h: setattr(mod, "_hook", h)
        sys.modules["antenv.axon_hooks"] = mod
        antenv.axon_hooks = mod
    except Exception:
        pass


def _host_prep(col, row, E):
    """Shared host prep. Returns (cores, cap, S, n_chunks, gathers)."""
    ec = E // NCORES
    cores = [_prep_core(col[k * ec:(k + 1) * ec], row[k * ec:(k + 1) * ec])
             for k in range(NCORES)]
    cap = {}
    for name in ("lo", "hi"):
        mx = max(len(cr[name][0]) for cr in cores)
        cap[name] = max(BATCH, ((mx + BATCH - 1) // BATCH) * BATCH)
    S = cap["lo"] + cap["hi"]
    n_chunks = S // CHUNK
    # shared (across cores) real-index count per gather instruction
    real = {nm: [len(cr[nm][0]) for cr in cores] for nm in ("lo", "hi")}
    gathers = []
    for pos0 in range(0, cap["lo"], BATCH):
        mx = max(min(L - pos0, BATCH) for L in real["lo"])
        n_reg = min(BATCH, ((max(mx, 0) + 15) // 16) * 16)
        if n_reg > 0:
            gathers.append((pos0, n_reg, 0))
    for pos0 in range(cap["lo"], S, BATCH):
        q0 = pos0 - cap["lo"]
        mx = max(min(L - q0, BATCH) for L in real["hi"])
        n_reg = min(BATCH, ((max(mx, 0) + 15) // 16) * 16)
        if n_reg > 0:
            gathers.append((pos0, n_reg, 1))
    return cores, cap, S, n_chunks, gathers


def kernel(emb, edge_index, W1, b1, W2, b2):
    global LAST_RESULTS
    from concourse.bass_utils import run_bass_kernel_spmd
    _ensure_ntff_hook()

    emb = np.asarray(emb, dtype=np.float32)
    W1 = np.asarray(W1, dtype=np.float32)
    W2 = np.asarray(W2, dtype=np.float32)
    b1 = np.asarray(b1, dtype=np.float32)
    b2 = np.asarray(b2, dtype=np.float32)
    assert np.abs(b1).max() == 0 and np.abs(b2).max() == 0, \
        "nonzero biases not implemented"
    col = np.asarray(edge_index[0]).astype(np.int64)
    row = np.asarray(edge_index[1]).astype(np.int64)

    n, h = emb.shape
    assert h == H
    E = col.shape[0]
    ec = E // NCORES
    n_pad = ((n + P - 1) // P) * P

    cores, cap, S, n_chunks, gathers = _host_prep(col, row, E)

    key = (n_pad, n_chunks, tuple(gathers))
    if key not in _PROG_CACHE:
        _PROG_CACHE[key] = _build_program(n_pad, n_chunks, gathers)
    nc = _PROG_CACHE[key]

    embT = np.zeros((P, n_pad), dtype=BF16)
    embT[:, :n] = emb.T.astype(BF16)
    w1t = W1.T.astype(BF16)
    w2t = W2.T.astype(BF16)

    shared = {"lo": {}, "hi": {}}
    for (pos0, n_reg, hf) in gathers:
        nm = "lo" if hf == 0 else "hi"
        b0 = pos0 if hf == 0 else pos0 - cap["lo"]
        shared[nm][b0] = n_reg
    in_maps = []
    opos_all = []
    for cr in cores:
        lc, lr, lm, lp = _pad_region(*cr["lo"], cap["lo"], shared["lo"])
        hc, hr, hm, hp = _pad_region(*cr["hi"], cap["hi"], shared["hi"])
        codes = np.concatenate([lc, hc])
        ridx = np.concatenate([lr, hr])
        meta = np.concatenate([lm, hm])
        opos = np.concatenate([lp, hp])
        # onehot blocks: oh[d, (c, s, e)] = 1 iff codes[c*CHUNK+e] == 128*s+d
        cd = codes.reshape(-1, CHUNK).astype(np.int32)      # [n_chunks, CHUNK]
        ohs = np.zeros((P, n_chunks, NSLOT, CHUNK), dtype=BF16)
        ccp, eep = np.meshgrid(np.arange(n_chunks), np.arange(CHUNK),
                               indexing="ij")
        vv = cd < 128 * NSLOT
        ohs[cd[vv] & 127, ccp[vv], cd[vv] >> 7, eep[vv]] = 1
        in_maps.append({
            "embT": embT, "w1t": w1t, "w2t": w2t,
            "oh": ohs.reshape(P, -1),
            "ridx": _wrap_idx(ridx),
            "meta": meta.reshape(1, -1).astype(np.uint32),
        })
        opos_all.append(opos)

    res = run_bass_kernel_spmd(nc, in_maps, core_ids=list(range(NCORES)))
    LAST_RESULTS = res

    # ---- host post: normalize + unpermute ----
    out = np.empty(E, dtype=np.float32)
    for k in range(NCORES):
        r = res.results[k]
        ssq = np.asarray(r["ssq"], dtype=np.float32)
        nrm = np.maximum(np.sqrt(ssq.T.reshape(-1)[:n]), 1e-8)
        dots = np.asarray(r["out"], dtype=np.float32).reshape(-1)[:S]
        opos = opos_all[k]
        valid = opos >= 0
        seg = out[k * ec:(k + 1) * ec]
        seg[opos[valid]] = dots[valid]
        cseg = col[k * ec:(k + 1) * ec]
        rseg = row[k * ec:(k + 1) * ec]
        seg /= nrm[cseg] * nrm[rseg]
    return out
